# revision 27
# baseline (speedup 1.0000x reference)
"""Multi-head masked self-attention on 8 TRN2 NeuronCores.

Problem: B=4, S=2048, D=1024, H=16 heads (hd=64), fp32 in/out.
  q,k,v = x@W* + b*; causal softmax(q k^T / 8) @ v; out = ctx @ Wo + bo.

Sharding: core c -> (batch b = c//2, head-group g = c%2 of 8 heads).
Each core computes a partial output projection over its 512 hidden dims;
the host sums the two partials per batch and adds bo.

Key layout/perf choices (v2, evolved from the fp32r baseline):
  - ALL matmul operands are float16 (inputs quantized host-side): PE runs
    1 elem/cell/cycle like bf16, weight loads hit the fast-weight-load
    path (~27ns vs ~187ns for fp32r 4-byte loads), DMA+SBUF traffic
    halves, and the quantization error (~1e-3 rel) is far inside the
    2e-2 gate.  PSUM accumulation stays fp32.
  - host passes xT = x[b].T; q^T, k^T computed directly as [512, S]
    (lhsT = W chunk, rhs = xT chunk); v in natural [S, 512] layout with
    a ones-column per head so the AV matmul accumulates the softmax
    denominator in PSUM row 64 for free.
  - scores are computed transposed st[sk, sq] = k q^T with K = hd = 64:
    the two heads of a column-group live in partition halves 0:64/64:128,
    so a PAIR of 64x128 PE row-tiles (tile_position (0,0)/(64,0))
    computes both heads' score blocks CONCURRENTLY in one N=512 slot,
    writing the two PSUM banks of one [128,2,512] tile.  exp runs as a
    single 1024-elem ACT call over both banks.
  - causal masking by multiplying 0/1 masks on only the sub-range of
    columns that can be masked; fully-masked blocks never computed; the
    two 256-wide diagonal chunks pack into one PSUM tile (one exp call).
  - q/k bias-add evacuations run on DVE (tensor_scalar_add), not ACT,
    keeping ACT exclusively for the exp stream that paces late waves.
  - ctx^T aliases qT's storage; output projection C(sq) groups are woven
    into the following wave so the kernel does not end with a serial
    projection tail; po is written fp16 and summed host-side.
"""

import numpy as np

import concourse.bass as bass
import concourse.mybir as mybir
import concourse.tile as tile
from concourse import bacc
from concourse.bass import ts
from concourse.bass_utils import run_bass_kernel_spmd

F32 = mybir.dt.float32
F16 = mybir.dt.float16
AF = mybir.ActivationFunctionType

B, S, D, H, HD = 4, 2048, 1024, 16, 64
G = 2                 # head groups (cores per batch)
DH = D // G           # hidden dims per core = 512
HPC = H // G          # heads per core = 8
NCORES = 8

NSQ = S // 512        # 4 sq tiles of 512
NSK = S // 128        # 16 sk chunks of 128
NFC = D // 128        # 8 feature chunks
NOC = DH // 128       # 4 out-dim chunks of the per-core hidden

WARMUP = 24           # HAM pre-warm matmuls while first DMAs land
QUAD = False          # 4-head groups: col-tiled AV pairs + 4x-tiled den
DEBUG_DUMP = False    # add kT/vA/ctx/den debug outputs to the program


def _mm(nc, out, lhsT, rhs, start, stop, tile_position=None, skip=False):
    nc.tensor.matmul(out, lhsT, rhs, start=start, stop=stop,
                     tile_position=tile_position, skip_group_check=skip)


def build_program():
    nc = bacc.Bacc("TRN2", target_bir_lowering=False, debug=False)

    xT_d = nc.dram_tensor("xT", [D, S], F16, kind="ExternalInput").ap()
    wq_d = nc.dram_tensor("wq", [D, DH], F16, kind="ExternalInput").ap()
    wk_d = nc.dram_tensor("wk", [D, DH], F16, kind="ExternalInput").ap()
    wv_d = nc.dram_tensor("wv", [D, DH], F16, kind="ExternalInput").ap()
    wo_d = nc.dram_tensor("wo", [DH, D], F16, kind="ExternalInput").ap()
    bqt_d = nc.dram_tensor("bqt", [128, NOC], F32, kind="ExternalInput").ap()
    bkt_d = nc.dram_tensor("bkt", [128, NOC], F32, kind="ExternalInput").ap()
    bvb_d = nc.dram_tensor("bvb", [128, HPC, HD], F32,
                           kind="ExternalInput").ap()
    mask_d = nc.dram_tensor("masks", [128, 3, 2, 512], F16,
                            kind="ExternalInput").ap()
    po_d = nc.dram_tensor("po", [S, D], F16, kind="ExternalOutput").ap()

    dbg = None
    if DEBUG_DUMP:
        dbg = {
            "kTo": nc.dram_tensor("kTo", [128, NOC, S], F16,
                                  kind="ExternalOutput").ap(),
            "vAo": nc.dram_tensor("vAo", [128, NSK, HPC, HD + 1], F16,
                                  kind="ExternalOutput").ap(),
            "ctxo": nc.dram_tensor("ctxo", [128, NOC, S], F16,
                                   kind="ExternalOutput").ap(),
            "deno": nc.dram_tensor("deno", [HPC, S], F32,
                                   kind="ExternalOutput").ap(),
            "qTo": nc.dram_tensor("qTo", [128, NOC, S], F16,
                                  kind="ExternalOutput").ap(),
        }

    with tile.TileContext(nc) as tc:
        _emit(tc, xT_d, wq_d, wk_d, wv_d, wo_d, bqt_d, bkt_d, bvb_d, mask_d,
              po_d, dbg)
    nc.compile()
    return nc


def _emit(tc, xT_d, wq_d, wk_d, wv_d, wo_d, bqt_d, bkt_d, bvb_d, mask_d,
          po_d, dbg=None):
    nc = tc.nc
    PS = bass.MemorySpace.PSUM

    with (
        tc.tile_pool(name="persist", bufs=1) as persist,
        tc.tile_pool(name="qkv", bufs=1) as qkv_pool,
        tc.tile_pool(name="exp", bufs=4) as exp_pool,
        tc.tile_pool(name="small", bufs=2) as small_pool,
        tc.tile_pool(name="wtsA", bufs=1) as wtsA,
        tc.tile_pool(name="xin", bufs=2) as xin,
        tc.tile_pool(name="woC", bufs=1) as woC,
        tc.tile_pool(name="poC", bufs=3) as poC,
        tc.tile_pool(name="ps_mm", bufs=(1 if QUAD else 2),
                     space=PS) as ps_mm,
        tc.tile_pool(name="ps_att", bufs=2, space=PS) as ps_att,  # 4 banks
        tc.tile_pool(name="ps_ctx", bufs=2, space=PS) as ps_ctx,  # 2 banks
        tc.tile_pool(name="ps_den", bufs=1, space=PS) as ps_den,  # 1 bank
    ):
        bqt = persist.tile([128, NOC], F32)
        bkt = persist.tile([128, NOC], F32)
        nc.sync.dma_start(bqt[:], bqt_d[:])
        nc.sync.dma_start(bkt[:], bkt_d[:])

        # HAM pre-warm: a short burst of throwaway matmuls on zeros while
        # the first x/wq DMAs land, so the PE clock ramps toward 2.4 GHz
        zw = persist.tile([128, 512], F16)
        nc.vector.memset(zw[:], 0.0)
        for i in range(WARMUP):
            pwarm = ps_mm.tile([128, 512], F32, name="mm")
            _mm(nc, pwarm[:], zw[:, 0:128], zw[:], True, True)

        # persistent activations.  qT doubles as ctx^T storage: wave j's
        # evacuation overwrites qT[:, :, j-tile] right after the last
        # score matmul that reads it (disjoint partition rows per head).
        qT = qkv_pool.tile([128, NOC, S], F16)        # q^T + bq, then ctx^T
        kT = qkv_pool.tile([128, NOC, S], F16)        # k^T + bk   [512, S]
        vA = qkv_pool.tile([128, NSK, HPC, HD + 1], F16)   # v + ones col
        nc.vector.memset(vA[:, :, :, HD:HD + 1], 1.0)
        ones_sb = persist.tile([128, 1], F16)
        nc.vector.memset(ones_sb[:], 1.0)
        # debug mode: don't alias ctx onto qT so q can be inspected
        if dbg is not None:
            cT = qkv_pool.tile([128, NOC, S], F16, name="cT")
        else:
            cT = qT

        # weights + first x tile; interleave x/wq chunks so the first
        # accumulation chain starts as soon as chunk 0 lands
        xts = {}
        xts[0] = xin.tile([128, NFC, 512], F16, name="xt")
        wq = wtsA.tile([128, NFC, DH], F16)
        wk = wtsA.tile([128, NFC, DH], F16)
        wv = wtsA.tile([128, NFC, DH], F16)
        for fc in range(NFC):
            nc.sync.dma_start(xts[0][:, fc], xT_d[ts(fc, 128), ts(0, 512)])
            nc.sync.dma_start(wq[:, fc], wq_d[ts(fc, 128), :])
        for fc in range(NFC):
            nc.sync.dma_start(wk[:, fc], wk_d[ts(fc, 128), :])
        for fc in range(NFC):
            nc.sync.dma_start(wv[:, fc], wv_d[ts(fc, 128), :])
        bvb = persist.tile([128, HPC, HD], F32)
        nc.sync.dma_start(bvb[:], bvb_d[:])
        mt = persist.tile([128, 3, 2, 512], F16)
        nc.sync.dma_start(mt[:], mask_d[:])
        wo = woC.tile([128, NOC, D], F16)
        for hc in range(NOC):
            nc.sync.dma_start(wo[:, hc], wo_d[ts(hc, 128), :])

        def emit_A_group(j, g):
            """One projection group of stage A(j): g=0..3 q/k half-pairs,
            g=4..5 v pairs.  One PSUM bank per half."""
            xt = xts[j]
            if g < 4:
                op, is_k = g // 2, g % 2
                wt, bias, dstT = ((wk, bkt, kT) if is_k else (wq, bqt, qT))
                for half in range(2):
                    oc = 2 * op + half
                    pt = ps_mm.tile([128, 512], F32, name="mm")
                    for fc in range(NFC):
                        _mm(nc, pt[:], wt[:, fc, ts(oc, 128)], xt[:, fc],
                            fc == 0, fc == NFC - 1)
                    if j < 3:
                        # waves 0-2: ACT has slack and this keeps the DVE
                        # FIFO from stalling the next A-group's matmuls
                        nc.scalar.activation(dstT[:, oc, ts(j, 512)],
                                             pt[:], AF.Identity,
                                             bias=bias[:, oc:oc + 1])
                    else:
                        # A(3) is emitted while ACT is exp-saturated
                        nc.vector.tensor_scalar_add(dstT[:, oc, ts(j, 512)],
                                                    pt[:],
                                                    bias[:, oc:oc + 1])
            else:
                sp = g - 4
                for half in range(2):
                    sc = 2 * sp + half
                    pv = ps_mm.tile([128, 512], F32, name="mm")
                    for fc in range(NFC):
                        _mm(nc, pv[:], xt[:, fc, ts(sc, 128)], wv[:, fc],
                            fc == 0, fc == NFC - 1)
                    pv_r = pv[:].rearrange("p (h u) -> p h u", u=HD)
                    nc.vector.tensor_add(vA[:, 4 * j + sc, :, 0:HD],
                                         pv_r, bvb[:])

        def emit_pair(hc, j):
            """Attention for head pair (2hc, 2hc+1) on sq tile j.
            Scores run as concurrent 64x128 PE row-tiles (one per head),
            AV accumulates ctx^T (+denominator row) per head."""
            hA, hB = 2 * hc, 2 * hc + 1
            qA = qT[0:64, hc, ts(j, 512)]
            qB = qT[64:128, hc, ts(j, 512)]
            ctxA = ps_ctx.tile([65, 512], F32, name="ctx")
            ctxB = ps_ctx.tile([65, 512], F32, name="ctx")
            nfull = 4 * j + 2
            for ik in range(nfull):
                stp = ps_att.tile([128, 2, 512], F32, name="att")
                _mm(nc, stp[:, 0, :], kT[0:64, hc, ts(ik, 128)], qA,
                    True, True, tile_position=(0, 0))
                _mm(nc, stp[:, 1, :], kT[64:128, hc, ts(ik, 128)], qB,
                    True, True, tile_position=(64, 0))
                e = exp_pool.tile([128, 2, 512], F16, name="e")
                nc.scalar.activation(e[:], stp[:], AF.Exp, scale=0.125)
                t = ik - 4 * j
                if t == 0:
                    # only sq columns 0:128 can be masked for this chunk
                    nc.vector.tensor_mul(e[:, :, 0:128], e[:, :, 0:128],
                                         mt[:, 0, :, 0:128])
                elif t == 1:
                    nc.vector.tensor_mul(e[:, :, 0:256], e[:, :, 0:256],
                                         mt[:, 1, :, 0:256])
                _mm(nc, ctxA[:], vA[:, ik, hA, :], e[:, 0, :],
                    ik == 0, False)
                _mm(nc, ctxB[:], vA[:, ik, hB, :], e[:, 1, :],
                    ik == 0, False)
            # two 256-wide diagonal chunks (sq columns 256:512 only),
            # packed side by side into one PSUM tile -> one exp call
            ik2, ik3 = 4 * j + 2, 4 * j + 3
            qA2 = qA[:, 256:512]
            qB2 = qB[:, 256:512]
            stp = ps_att.tile([128, 2, 512], F32, name="att")
            _mm(nc, stp[:, 0, 0:256], kT[0:64, hc, ts(ik2, 128)], qA2,
                True, True, tile_position=(0, 0))
            _mm(nc, stp[:, 1, 0:256], kT[64:128, hc, ts(ik2, 128)], qB2,
                True, True, tile_position=(64, 0))
            _mm(nc, stp[:, 0, 256:512], kT[0:64, hc, ts(ik3, 128)], qA2,
                True, True, tile_position=(0, 0))
            _mm(nc, stp[:, 1, 256:512], kT[64:128, hc, ts(ik3, 128)], qB2,
                True, True, tile_position=(64, 0))
            e = exp_pool.tile([128, 2, 512], F16, name="e")
            nc.scalar.activation(e[:], stp[:], AF.Exp, scale=0.125)
            nc.vector.tensor_mul(e[:], e[:], mt[:, 2])
            _mm(nc, ctxA[:, 256:512], vA[:, ik2, hA, :], e[:, 0, 0:256],
                False, False)
            _mm(nc, ctxB[:, 256:512], vA[:, ik2, hB, :], e[:, 1, 0:256],
                False, False)
            _mm(nc, ctxA[:, 256:512], vA[:, ik3, hA, :], e[:, 0, 256:512],
                False, True)
            _mm(nc, ctxB[:, 256:512], vA[:, ik3, hB, :], e[:, 1, 256:512],
                False, True)
            # normalize straight out of PSUM into qT's freed j-tile.
            # NB: reciprocal_approx_fast drops nonzero partition offsets on
            # HW and PSUM reads must be 32-aligned, so copy the den row
            # (partition 64) to a partition-0 tile first.
            for ctx, hp in ((ctxA, 0), (ctxB, 64)):
                dst = cT[hp:hp + 64, hc, ts(j, 512)]
                denb = small_pool.tile([1, 512], F32, name="denb")
                nc.vector.tensor_copy(denb[:], ctx[64:65, :])
                rec = small_pool.tile([1, 512], F32, name="rec")
                nc.vector.reciprocal_approx_fast(rec[:], denb[:])
                if dbg is not None:
                    h = 2 * hc + (1 if hp else 0)
                    nc.sync.dma_start(dbg["deno"][h:h + 1, ts(j, 512)],
                                      rec[:])
                bcs = small_pool.tile([64, 512], F32, name="bcs")
                nc.gpsimd.partition_broadcast(bcs[:], rec[:])
                nc.vector.tensor_mul(dst, ctx[0:64, :], bcs[:])

        def emit_quad(qd, j):
            """Attention for 4 heads (column groups 2qd, 2qd+1) on sq tile
            j.  Scores: concurrent 64x128 row-tile pairs per column group.
            AV: col-tiled head pairs share one ctx bank (partitions
            0:64/64:128).  Denominators: four concurrent M=1 col-tiled
            matmuls (positions 0/32/64/96) accumulate into one den bank."""
            hcA, hcB = 2 * qd, 2 * qd + 1
            ctxAB = ps_ctx.tile([128, 512], F32, name="ctx")
            ctxCD = ps_ctx.tile([128, 512], F32, name="ctx")
            den = ps_den.tile([128, 512], F32, name="den")
            qs = {hcA: (qT[0:64, hcA, ts(j, 512)], qT[64:128, hcA, ts(j, 512)]),
                  hcB: (qT[0:64, hcB, ts(j, 512)], qT[64:128, hcB, ts(j, 512)])}

            def scores_exp(hc, ik):
                qA, qB = qs[hc]
                stp = ps_att.tile([128, 2, 512], F32, name="att")
                _mm(nc, stp[:, 0, :], kT[0:64, hc, ts(ik, 128)], qA,
                    True, True, tile_position=(0, 0))
                _mm(nc, stp[:, 1, :], kT[64:128, hc, ts(ik, 128)], qB,
                    True, True, tile_position=(64, 0))
                e = exp_pool.tile([128, 2, 512], F16, name="e")
                nc.scalar.activation(e[:], stp[:], AF.Exp, scale=0.125)
                t = ik - 4 * j
                if t == 0:
                    nc.vector.tensor_mul(e[:, :, 0:128], e[:, :, 0:128],
                                         mt[:, 0, :, 0:128])
                elif t == 1:
                    nc.vector.tensor_mul(e[:, :, 0:256], e[:, :, 0:256],
                                         mt[:, 1, :, 0:256])
                return e

            nfull = 4 * j + 2
            for ik in range(nfull):
                e1 = scores_exp(hcA, ik)
                e2 = scores_exp(hcB, ik)
                st = ik == 0
                _mm(nc, ctxAB[0:64, :], vA[:, ik, 2 * hcA, 0:HD],
                    e1[:, 0, :], st, False, tile_position=(0, 0), skip=True)
                _mm(nc, ctxAB[64:128, :], vA[:, ik, 2 * hcA + 1, 0:HD],
                    e1[:, 1, :], st, False, tile_position=(0, 64), skip=True)
                _mm(nc, ctxCD[0:64, :], vA[:, ik, 2 * hcB, 0:HD],
                    e2[:, 0, :], st, False, tile_position=(0, 0), skip=True)
                _mm(nc, ctxCD[64:128, :], vA[:, ik, 2 * hcB + 1, 0:HD],
                    e2[:, 1, :], st, False, tile_position=(0, 64), skip=True)
                for m, es in ((0, e1[:, 0, :]), (1, e1[:, 1, :]),
                              (2, e2[:, 0, :]), (3, e2[:, 1, :])):
                    _mm(nc, den[32 * m:32 * m + 1, :], ones_sb[:], es,
                        st, False, tile_position=(0, 32 * m), skip=True)

            # two 256-wide diagonal chunks packed per column group
            ik2, ik3 = 4 * j + 2, 4 * j + 3
            ehi = {}
            for hc in (hcA, hcB):
                qA, qB = qs[hc]
                qA2, qB2 = qA[:, 256:512], qB[:, 256:512]
                stp = ps_att.tile([128, 2, 512], F32, name="att")
                _mm(nc, stp[:, 0, 0:256], kT[0:64, hc, ts(ik2, 128)], qA2,
                    True, True, tile_position=(0, 0))
                _mm(nc, stp[:, 1, 0:256], kT[64:128, hc, ts(ik2, 128)], qB2,
                    True, True, tile_position=(64, 0))
                _mm(nc, stp[:, 0, 256:512], kT[0:64, hc, ts(ik3, 128)], qA2,
                    True, True, tile_position=(0, 0))
                _mm(nc, stp[:, 1, 256:512], kT[64:128, hc, ts(ik3, 128)],
                    qB2, True, True, tile_position=(64, 0))
                e = exp_pool.tile([128, 2, 512], F16, name="e")
                nc.scalar.activation(e[:], stp[:], AF.Exp, scale=0.125)
                nc.vector.tensor_mul(e[:], e[:], mt[:, 2])
                ehi[hc] = e
            e1, e2 = ehi[hcA], ehi[hcB]
            for ika, sl in ((ik2, slice(0, 256)), (ik3, slice(256, 512))):
                last = ika == ik3
                _mm(nc, ctxAB[0:64, 256:512], vA[:, ika, 2 * hcA, 0:HD],
                    e1[:, 0, sl], False, last, tile_position=(0, 0),
                    skip=True)
                _mm(nc, ctxAB[64:128, 256:512], vA[:, ika, 2 * hcA + 1, 0:HD],
                    e1[:, 1, sl], False, last, tile_position=(0, 64), skip=True)
                _mm(nc, ctxCD[0:64, 256:512], vA[:, ika, 2 * hcB, 0:HD],
                    e2[:, 0, sl], False, last, tile_position=(0, 0),
                    skip=True)
                _mm(nc, ctxCD[64:128, 256:512], vA[:, ika, 2 * hcB + 1, 0:HD],
                    e2[:, 1, sl], False, last, tile_position=(0, 64), skip=True)
                for m, es in ((0, e1[:, 0, sl]), (1, e1[:, 1, sl]),
                              (2, e2[:, 0, sl]), (3, e2[:, 1, sl])):
                    _mm(nc, den[32 * m:32 * m + 1, 256:512], ones_sb[:], es,
                        False, last, tile_position=(0, 32 * m), skip=True)

            for m, (ctxt, hc, hp) in enumerate((
                    (ctxAB[0:64, :], hcA, 0), (ctxAB[64:128, :], hcA, 64),
                    (ctxCD[0:64, :], hcB, 0), (ctxCD[64:128, :], hcB, 64))):
                dst = cT[hp:hp + 64, hc, ts(j, 512)]
                denb = small_pool.tile([1, 512], F32, name="denb")
                nc.vector.tensor_copy(denb[:], den[32 * m:32 * m + 1, :])
                rec = small_pool.tile([1, 512], F32, name="rec")
                nc.vector.reciprocal_approx_fast(rec[:], denb[:])
                if dbg is not None:
                    h = 2 * hc + (1 if hp else 0)
                    nc.sync.dma_start(dbg["deno"][h:h + 1, ts(j, 512)],
                                      rec[:])
                bcs = small_pool.tile([64, 512], F32, name="bcs")
                nc.gpsimd.partition_broadcast(bcs[:], rec[:])
                nc.vector.tensor_mul(dst, ctxt, bcs[:])

        def emit_C_group(sq):
            """Output projection for one 128-row sq chunk."""
            ot = poC.tile([128, 2, 512], F16, name="ot")
            for oc in range(2):
                pp = ps_mm.tile([128, 512], F32, name="mm")
                for hcc in range(NOC):
                    _mm(nc, pp[:], cT[:, hcc, ts(sq, 128)],
                        wo[:, hcc, ts(oc, 512)],
                        hcc == 0, hcc == NOC - 1)
                if sq < 8:
                    # C(0..7) run during waves 1-2 where ACT has slack
                    nc.scalar.activation(ot[:, oc, :], pp[:], AF.Copy)
                else:
                    nc.vector.tensor_copy(ot[:, oc, :], pp[:])
            nc.sync.dma_start(po_d[ts(sq, 128), :],
                              ot[:].rearrange("p a b -> p (a b)"))

        # A(j) projections, then attention wave j with the previous
        # wave's output-projection groups woven between head pairs
        for j in range(NSQ):
            if j > 0:
                xts[j] = xin.tile([128, NFC, 512], F16, name="xt")
                for fc in range(NFC):
                    nc.sync.dma_start(xts[j][:, fc],
                                      xT_d[ts(fc, 128), ts(j, 512)])
            for g in range(6):
                emit_A_group(j, g)
            if QUAD:
                for qd in range(2):
                    emit_quad(qd, j)
                    if j > 0:
                        emit_C_group(4 * (j - 1) + 2 * qd)
                        emit_C_group(4 * (j - 1) + 2 * qd + 1)
            else:
                for hc in range(NOC):
                    emit_pair(hc, j)
                    if j > 0:
                        emit_C_group(4 * (j - 1) + hc)
        # the last pair's normalize chain (copy/recip -> gpsimd broadcast
        # -> mul) idles the PE long enough to re-throttle the clock; a few
        # fillers keep it at 2.4 GHz so the C tail runs warm
        for i in range(6):
            fill = ps_mm.tile([128, 512], F32, name="mm")
            _mm(nc, fill[:], zw[:, 0:128], zw[:], True, True)
        for sq in range(12, 16):
            emit_C_group(sq)
        if dbg is not None:
            nc.sync.dma_start(dbg["kTo"][:], kT[:])
            nc.sync.dma_start(dbg["vAo"][:], vA[:])
            nc.sync.dma_start(dbg["ctxo"][:], cT[:])
            nc.sync.dma_start(dbg["qTo"][:], qT[:])


def make_masks():
    p = np.arange(128)[:, None]
    c = np.arange(512)[None, :]
    m0 = (c >= p).astype(np.float16)
    m1 = (c >= p + 128).astype(np.float16)
    t23 = np.concatenate([m0[:, 0:256], m1[:, 0:256]], axis=1)
    m = np.empty((128, 3, 2, 512), dtype=np.float16)
    for dd in range(2):
        m[:, 0, dd] = m0
        m[:, 1, dd] = m1
        m[:, 2, dd] = t23
    return m


def make_in_maps(x, Wq, bq, Wk, bk, Wv, bv, Wo):
    masks = make_masks()
    in_maps = []
    for c in range(NCORES):
        b, g = c // 2, c % 2
        sl = slice(g * DH, (g + 1) * DH)
        in_maps.append({
            "xT": np.ascontiguousarray(x[b].T.astype(np.float16)),
            "wq": np.ascontiguousarray(Wq[:, sl].astype(np.float16)),
            "wk": np.ascontiguousarray(Wk[:, sl].astype(np.float16)),
            "wv": np.ascontiguousarray(Wv[:, sl].astype(np.float16)),
            "wo": np.ascontiguousarray(Wo[sl, :].astype(np.float16)),
            "bqt": np.ascontiguousarray(bq[sl].reshape(NOC, 128).T),
            "bkt": np.ascontiguousarray(bk[sl].reshape(NOC, 128).T),
            "bvb": np.ascontiguousarray(
                np.broadcast_to(bv[sl].reshape(HPC, HD), (128, HPC, HD))),
            "masks": masks,
        })
    return in_maps


_CACHE = {}


def _get_program():
    if "prog" not in _CACHE:
        _CACHE["prog"] = build_program()
    return _CACHE["prog"]


def kernel(x, Wq, bq, Wk, bk, Wv, bv, Wo, bo, **run_kwargs):
    x = np.asarray(x, dtype=np.float32)
    Wq = np.asarray(Wq, dtype=np.float32)
    bq = np.asarray(bq, dtype=np.float32)
    Wk = np.asarray(Wk, dtype=np.float32)
    bk = np.asarray(bk, dtype=np.float32)
    Wv = np.asarray(Wv, dtype=np.float32)
    bv = np.asarray(bv, dtype=np.float32)
    Wo = np.asarray(Wo, dtype=np.float32)
    bo = np.asarray(bo, dtype=np.float32)

    run_kwargs.pop("f32r", None)
    nc = _get_program()
    in_maps = make_in_maps(x, Wq, bq, Wk, bk, Wv, bv, Wo)
    res = run_bass_kernel_spmd(nc, in_maps, list(range(NCORES)), **run_kwargs)
    out = np.empty((B, S, D), dtype=np.float32)
    for b in range(B):
        out[b] = (res.results[2 * b]["po"].astype(np.float32)
                  + res.results[2 * b + 1]["po"].astype(np.float32) + bo)
    _CACHE["last_results"] = res
    return out


# revision 28
# speedup vs baseline: 1.2123x; 1.2123x over previous
"""Multi-head masked self-attention on 8 TRN2 NeuronCores.

Problem: B=4, S=2048, D=1024, H=16 heads (hd=64), fp32 in/out.
  q,k,v = x@W* + b*; causal softmax(q k^T / 8) @ v; out = ctx @ Wo + bo.

Sharding: core c -> (batch b = c//2, head-group g = c%2 of 8 heads).
Each core computes a partial output projection over its 512 hidden dims;
the host sums the two partials per batch and adds bo.

Key layout/perf choices (v2, evolved from the fp32r baseline):
  - ALL matmul operands are float16 (inputs quantized host-side): PE runs
    1 elem/cell/cycle like bf16, weight loads hit the fast-weight-load
    path (~27ns vs ~187ns for fp32r 4-byte loads), DMA+SBUF traffic
    halves, and the quantization error (~1e-3 rel) is far inside the
    2e-2 gate.  PSUM accumulation stays fp32.
  - host passes xT = x[b].T; q^T, k^T computed directly as [512, S]
    (lhsT = W chunk, rhs = xT chunk); v in natural [S, 512] layout with
    a ones-column per head so the AV matmul accumulates the softmax
    denominator in PSUM row 64 for free.
  - scores are computed transposed st[sk, sq] = k q^T with K = hd = 64:
    the two heads of a column-group live in partition halves 0:64/64:128,
    so a PAIR of 64x128 PE row-tiles (tile_position (0,0)/(64,0))
    computes both heads' score blocks CONCURRENTLY in one N=512 slot,
    writing the two PSUM banks of one [128,2,512] tile.  exp runs as a
    single 1024-elem ACT call over both banks.
  - causal masking by multiplying 0/1 masks on only the sub-range of
    columns that can be masked; fully-masked blocks never computed; the
    two 256-wide diagonal chunks pack into one PSUM tile (one exp call).
  - q/k bias-add evacuations run on DVE (tensor_scalar_add), not ACT,
    keeping ACT exclusively for the exp stream that paces late waves.
  - ctx^T aliases qT's storage; output projection C(sq) groups are woven
    into the following wave so the kernel does not end with a serial
    projection tail; po is written fp16 and summed host-side.
"""

import numpy as np

import concourse.bass as bass
import concourse.mybir as mybir
import concourse.tile as tile
from concourse import bacc
from concourse.bass import ts
from concourse.bass_utils import run_bass_kernel_spmd

F32 = mybir.dt.float32
F16 = mybir.dt.float16
AF = mybir.ActivationFunctionType

B, S, D, H, HD = 4, 2048, 1024, 16, 64
G = 2                 # head groups (cores per batch)
DH = D // G           # hidden dims per core = 512
HPC = H // G          # heads per core = 8
NCORES = 8

NSQ = S // 512        # 4 sq tiles of 512
NSK = S // 128        # 16 sk chunks of 128
NFC = D // 128        # 8 feature chunks
NOC = DH // 128       # 4 out-dim chunks of the per-core hidden

WARMUP = 24           # HAM pre-warm matmuls while first DMAs land
QUAD = False          # 4-head groups: col-tiled AV pairs + 4x-tiled den
DEBUG_DUMP = False    # add kT/vA/ctx/den debug outputs to the program


def _mm(nc, out, lhsT, rhs, start, stop, tile_position=None, skip=False):
    nc.tensor.matmul(out, lhsT, rhs, start=start, stop=stop,
                     tile_position=tile_position, skip_group_check=skip)


def build_program():
    nc = bacc.Bacc("TRN2", target_bir_lowering=False, debug=False)

    xT_d = nc.dram_tensor("xT", [D, S], F16, kind="ExternalInput").ap()
    wq_d = nc.dram_tensor("wq", [D, DH], F16, kind="ExternalInput").ap()
    wk_d = nc.dram_tensor("wk", [D, DH], F16, kind="ExternalInput").ap()
    wv_d = nc.dram_tensor("wv", [D, DH], F16, kind="ExternalInput").ap()
    wo_d = nc.dram_tensor("wo", [DH, D], F16, kind="ExternalInput").ap()
    bqt_d = nc.dram_tensor("bqt", [128, NOC], F32, kind="ExternalInput").ap()
    bkt_d = nc.dram_tensor("bkt", [128, NOC], F32, kind="ExternalInput").ap()
    bvb_d = nc.dram_tensor("bvb", [128, HPC, HD], F32,
                           kind="ExternalInput").ap()
    mask_d = nc.dram_tensor("masks", [128, 3, 2, 512], F16,
                            kind="ExternalInput").ap()
    po_d = nc.dram_tensor("po", [S, D], F16, kind="ExternalOutput").ap()

    dbg = None
    if DEBUG_DUMP:
        dbg = {
            "kTo": nc.dram_tensor("kTo", [128, NOC, S], F16,
                                  kind="ExternalOutput").ap(),
            "vAo": nc.dram_tensor("vAo", [128, NSK, HPC, HD + 1], F16,
                                  kind="ExternalOutput").ap(),
            "ctxo": nc.dram_tensor("ctxo", [128, NOC, S], F16,
                                   kind="ExternalOutput").ap(),
            "deno": nc.dram_tensor("deno", [HPC, S], F32,
                                   kind="ExternalOutput").ap(),
            "qTo": nc.dram_tensor("qTo", [128, NOC, S], F16,
                                  kind="ExternalOutput").ap(),
        }

    with tile.TileContext(nc) as tc:
        _emit(tc, xT_d, wq_d, wk_d, wv_d, wo_d, bqt_d, bkt_d, bvb_d, mask_d,
              po_d, dbg)
    nc.compile()
    return nc


def _emit(tc, xT_d, wq_d, wk_d, wv_d, wo_d, bqt_d, bkt_d, bvb_d, mask_d,
          po_d, dbg=None):
    nc = tc.nc
    PS = bass.MemorySpace.PSUM

    with (
        tc.tile_pool(name="persist", bufs=1) as persist,
        tc.tile_pool(name="qkv", bufs=1) as qkv_pool,
        tc.tile_pool(name="exp", bufs=4) as exp_pool,
        tc.tile_pool(name="small", bufs=2) as small_pool,
        tc.tile_pool(name="wtsA", bufs=1) as wtsA,
        tc.tile_pool(name="xin", bufs=2) as xin,
        tc.tile_pool(name="woC", bufs=1) as woC,
        tc.tile_pool(name="poC", bufs=3) as poC,
        tc.tile_pool(name="ps_mm", bufs=(1 if QUAD else 2),
                     space=PS) as ps_mm,
        tc.tile_pool(name="ps_att", bufs=2, space=PS) as ps_att,  # 4 banks
        tc.tile_pool(name="ps_ctx", bufs=2, space=PS) as ps_ctx,  # 2 banks
        tc.tile_pool(name="ps_den", bufs=1, space=PS) as ps_den,  # 1 bank
    ):
        bqt = persist.tile([128, NOC], F32)
        bkt = persist.tile([128, NOC], F32)
        nc.sync.dma_start(bqt[:], bqt_d[:])
        nc.sync.dma_start(bkt[:], bkt_d[:])

        # HAM pre-warm: a short burst of throwaway matmuls on zeros while
        # the first x/wq DMAs land, so the PE clock ramps toward 2.4 GHz
        zw = persist.tile([128, 512], F16)
        nc.vector.memset(zw[:], 0.0)
        for i in range(WARMUP):
            pwarm = ps_mm.tile([128, 512], F32, name="mm")
            _mm(nc, pwarm[:], zw[:, 0:128], zw[:], True, True)

        # persistent activations.  qT doubles as ctx^T storage: wave j's
        # evacuation overwrites qT[:, :, j-tile] right after the last
        # score matmul that reads it (disjoint partition rows per head).
        qT = qkv_pool.tile([128, NOC, S], F16)        # q^T + bq, then ctx^T
        kT = qkv_pool.tile([128, NOC, S], F16)        # k^T + bk   [512, S]
        vA = qkv_pool.tile([128, NSK, HPC, HD + 1], F16)   # v + ones col
        nc.vector.memset(vA[:, :, :, HD:HD + 1], 1.0)
        ones_sb = persist.tile([128, 1], F16)
        nc.vector.memset(ones_sb[:], 1.0)
        # debug mode: don't alias ctx onto qT so q can be inspected
        if dbg is not None:
            cT = qkv_pool.tile([128, NOC, S], F16, name="cT")
        else:
            cT = qT

        # weights + first x tile; interleave x/wq chunks so the first
        # accumulation chain starts as soon as chunk 0 lands
        xts = {}
        xts[0] = xin.tile([128, NFC, 512], F16, name="xt")
        wq = wtsA.tile([128, NFC, DH], F16)
        wk = wtsA.tile([128, NFC, DH], F16)
        wv = wtsA.tile([128, NFC, DH], F16)
        for fc in range(NFC):
            nc.sync.dma_start(xts[0][:, fc], xT_d[ts(fc, 128), ts(0, 512)])
            nc.sync.dma_start(wq[:, fc], wq_d[ts(fc, 128), :])
        for fc in range(NFC):
            nc.sync.dma_start(wk[:, fc], wk_d[ts(fc, 128), :])
        for fc in range(NFC):
            nc.sync.dma_start(wv[:, fc], wv_d[ts(fc, 128), :])
        bvb = persist.tile([128, HPC, HD], F32)
        nc.sync.dma_start(bvb[:], bvb_d[:])
        mt = persist.tile([128, 3, 2, 512], F16)
        nc.sync.dma_start(mt[:], mask_d[:])
        wo = woC.tile([128, NOC, D], F16)
        for hc in range(NOC):
            nc.sync.dma_start(wo[:, hc], wo_d[ts(hc, 128), :])

        def emit_A_group(j, g):
            """One projection group of stage A(j): g=0..3 q/k half-pairs,
            g=4..5 v pairs.  One PSUM bank per half."""
            xt = xts[j]
            if g < 4:
                op, is_k = g // 2, g % 2
                wt, bias, dstT = ((wk, bkt, kT) if is_k else (wq, bqt, qT))
                for half in range(2):
                    oc = 2 * op + half
                    pt = ps_mm.tile([128, 512], F32, name="mm")
                    for fc in range(NFC):
                        _mm(nc, pt[:], wt[:, fc, ts(oc, 128)], xt[:, fc],
                            fc == 0, fc == NFC - 1)
                    nc.vector.tensor_scalar_add(dstT[:, oc, ts(j, 512)],
                                                pt[:], bias[:, oc:oc + 1])
            else:
                sp = g - 4
                for half in range(2):
                    sc = 2 * sp + half
                    pv = ps_mm.tile([128, 512], F32, name="mm")
                    for fc in range(NFC):
                        _mm(nc, pv[:], xt[:, fc, ts(sc, 128)], wv[:, fc],
                            fc == 0, fc == NFC - 1)
                    pv_r = pv[:].rearrange("p (h u) -> p h u", u=HD)
                    nc.vector.tensor_add(vA[:, 4 * j + sc, :, 0:HD],
                                         pv_r, bvb[:])

        def emit_pair(hc, j):
            """Attention for head pair (2hc, 2hc+1) on sq tile j.
            Scores run as concurrent 64x128 PE row-tiles (one per head),
            AV accumulates ctx^T (+denominator row) per head."""
            hA, hB = 2 * hc, 2 * hc + 1
            qA = qT[0:64, hc, ts(j, 512)]
            qB = qT[64:128, hc, ts(j, 512)]
            ctxA = ps_ctx.tile([65, 512], F32, name="ctx")
            ctxB = ps_ctx.tile([65, 512], F32, name="ctx")
            nfull = 4 * j + 2
            for ik in range(nfull):
                stp = ps_att.tile([128, 2, 512], F32, name="att")
                _mm(nc, stp[:, 0, :], kT[0:64, hc, ts(ik, 128)], qA,
                    True, True, tile_position=(0, 0))
                _mm(nc, stp[:, 1, :], kT[64:128, hc, ts(ik, 128)], qB,
                    True, True, tile_position=(64, 0))
                e = exp_pool.tile([128, 2, 512], F16, name="e")
                nc.scalar.activation(e[:], stp[:], AF.Exp, scale=0.125)
                t = ik - 4 * j
                if t == 0:
                    # only sq columns 0:128 can be masked for this chunk
                    nc.vector.tensor_mul(e[:, :, 0:128], e[:, :, 0:128],
                                         mt[:, 0, :, 0:128])
                elif t == 1:
                    nc.vector.tensor_mul(e[:, :, 0:256], e[:, :, 0:256],
                                         mt[:, 1, :, 0:256])
                _mm(nc, ctxA[:], vA[:, ik, hA, :], e[:, 0, :],
                    ik == 0, False)
                _mm(nc, ctxB[:], vA[:, ik, hB, :], e[:, 1, :],
                    ik == 0, False)
            # two 256-wide diagonal chunks (sq columns 256:512 only),
            # packed side by side into one PSUM tile -> one exp call
            ik2, ik3 = 4 * j + 2, 4 * j + 3
            qA2 = qA[:, 256:512]
            qB2 = qB[:, 256:512]
            stp = ps_att.tile([128, 2, 512], F32, name="att")
            _mm(nc, stp[:, 0, 0:256], kT[0:64, hc, ts(ik2, 128)], qA2,
                True, True, tile_position=(0, 0))
            _mm(nc, stp[:, 1, 0:256], kT[64:128, hc, ts(ik2, 128)], qB2,
                True, True, tile_position=(64, 0))
            _mm(nc, stp[:, 0, 256:512], kT[0:64, hc, ts(ik3, 128)], qA2,
                True, True, tile_position=(0, 0))
            _mm(nc, stp[:, 1, 256:512], kT[64:128, hc, ts(ik3, 128)], qB2,
                True, True, tile_position=(64, 0))
            e = exp_pool.tile([128, 2, 512], F16, name="e")
            nc.scalar.activation(e[:], stp[:], AF.Exp, scale=0.125)
            nc.vector.tensor_mul(e[:], e[:], mt[:, 2])
            _mm(nc, ctxA[:, 256:512], vA[:, ik2, hA, :], e[:, 0, 0:256],
                False, False)
            _mm(nc, ctxB[:, 256:512], vA[:, ik2, hB, :], e[:, 1, 0:256],
                False, False)
            _mm(nc, ctxA[:, 256:512], vA[:, ik3, hA, :], e[:, 0, 256:512],
                False, True)
            _mm(nc, ctxB[:, 256:512], vA[:, ik3, hB, :], e[:, 1, 256:512],
                False, True)
            # normalize straight out of PSUM into qT's freed j-tile.
            # NB: reciprocal_approx_fast drops nonzero partition offsets on
            # HW and PSUM reads must be 32-aligned, so copy the den row
            # (partition 64) to a partition-0 tile first.
            for ctx, hp in ((ctxA, 0), (ctxB, 64)):
                dst = cT[hp:hp + 64, hc, ts(j, 512)]
                denb = small_pool.tile([1, 512], F32, name="denb")
                nc.vector.tensor_copy(denb[:], ctx[64:65, :])
                rec = small_pool.tile([1, 512], F32, name="rec")
                nc.vector.reciprocal_approx_fast(rec[:], denb[:])
                if dbg is not None:
                    h = 2 * hc + (1 if hp else 0)
                    nc.sync.dma_start(dbg["deno"][h:h + 1, ts(j, 512)],
                                      rec[:])
                bcs = small_pool.tile([64, 512], F32, name="bcs")
                nc.gpsimd.partition_broadcast(bcs[:], rec[:])
                nc.vector.tensor_mul(dst, ctx[0:64, :], bcs[:])

        def emit_quad(qd, j):
            """Attention for 4 heads (column groups 2qd, 2qd+1) on sq tile
            j.  Scores: concurrent 64x128 row-tile pairs per column group.
            AV: col-tiled head pairs share one ctx bank (partitions
            0:64/64:128).  Denominators: four concurrent M=1 col-tiled
            matmuls (positions 0/32/64/96) accumulate into one den bank."""
            hcA, hcB = 2 * qd, 2 * qd + 1
            ctxAB = ps_ctx.tile([128, 512], F32, name="ctx")
            ctxCD = ps_ctx.tile([128, 512], F32, name="ctx")
            den = ps_den.tile([128, 512], F32, name="den")
            qs = {hcA: (qT[0:64, hcA, ts(j, 512)], qT[64:128, hcA, ts(j, 512)]),
                  hcB: (qT[0:64, hcB, ts(j, 512)], qT[64:128, hcB, ts(j, 512)])}

            def scores_exp(hc, ik):
                qA, qB = qs[hc]
                stp = ps_att.tile([128, 2, 512], F32, name="att")
                _mm(nc, stp[:, 0, :], kT[0:64, hc, ts(ik, 128)], qA,
                    True, True, tile_position=(0, 0))
                _mm(nc, stp[:, 1, :], kT[64:128, hc, ts(ik, 128)], qB,
                    True, True, tile_position=(64, 0))
                e = exp_pool.tile([128, 2, 512], F16, name="e")
                nc.scalar.activation(e[:], stp[:], AF.Exp, scale=0.125)
                t = ik - 4 * j
                if t == 0:
                    nc.vector.tensor_mul(e[:, :, 0:128], e[:, :, 0:128],
                                         mt[:, 0, :, 0:128])
                elif t == 1:
                    nc.vector.tensor_mul(e[:, :, 0:256], e[:, :, 0:256],
                                         mt[:, 1, :, 0:256])
                return e

            nfull = 4 * j + 2
            for ik in range(nfull):
                e1 = scores_exp(hcA, ik)
                e2 = scores_exp(hcB, ik)
                st = ik == 0
                _mm(nc, ctxAB[0:64, :], vA[:, ik, 2 * hcA, 0:HD],
                    e1[:, 0, :], st, False, tile_position=(0, 0), skip=True)
                _mm(nc, ctxAB[64:128, :], vA[:, ik, 2 * hcA + 1, 0:HD],
                    e1[:, 1, :], st, False, tile_position=(0, 64), skip=True)
                _mm(nc, ctxCD[0:64, :], vA[:, ik, 2 * hcB, 0:HD],
                    e2[:, 0, :], st, False, tile_position=(0, 0), skip=True)
                _mm(nc, ctxCD[64:128, :], vA[:, ik, 2 * hcB + 1, 0:HD],
                    e2[:, 1, :], st, False, tile_position=(0, 64), skip=True)
                for m, es in ((0, e1[:, 0, :]), (1, e1[:, 1, :]),
                              (2, e2[:, 0, :]), (3, e2[:, 1, :])):
                    _mm(nc, den[32 * m:32 * m + 1, :], ones_sb[:], es,
                        st, False, tile_position=(0, 32 * m), skip=True)

            # two 256-wide diagonal chunks packed per column group
            ik2, ik3 = 4 * j + 2, 4 * j + 3
            ehi = {}
            for hc in (hcA, hcB):
                qA, qB = qs[hc]
                qA2, qB2 = qA[:, 256:512], qB[:, 256:512]
                stp = ps_att.tile([128, 2, 512], F32, name="att")
                _mm(nc, stp[:, 0, 0:256], kT[0:64, hc, ts(ik2, 128)], qA2,
                    True, True, tile_position=(0, 0))
                _mm(nc, stp[:, 1, 0:256], kT[64:128, hc, ts(ik2, 128)], qB2,
                    True, True, tile_position=(64, 0))
                _mm(nc, stp[:, 0, 256:512], kT[0:64, hc, ts(ik3, 128)], qA2,
                    True, True, tile_position=(0, 0))
                _mm(nc, stp[:, 1, 256:512], kT[64:128, hc, ts(ik3, 128)],
                    qB2, True, True, tile_position=(64, 0))
                e = exp_pool.tile([128, 2, 512], F16, name="e")
                nc.scalar.activation(e[:], stp[:], AF.Exp, scale=0.125)
                nc.vector.tensor_mul(e[:], e[:], mt[:, 2])
                ehi[hc] = e
            e1, e2 = ehi[hcA], ehi[hcB]
            for ika, sl in ((ik2, slice(0, 256)), (ik3, slice(256, 512))):
                last = ika == ik3
                _mm(nc, ctxAB[0:64, 256:512], vA[:, ika, 2 * hcA, 0:HD],
                    e1[:, 0, sl], False, last, tile_position=(0, 0),
                    skip=True)
                _mm(nc, ctxAB[64:128, 256:512], vA[:, ika, 2 * hcA + 1, 0:HD],
                    e1[:, 1, sl], False, last, tile_position=(0, 64), skip=True)
                _mm(nc, ctxCD[0:64, 256:512], vA[:, ika, 2 * hcB, 0:HD],
                    e2[:, 0, sl], False, last, tile_position=(0, 0),
                    skip=True)
                _mm(nc, ctxCD[64:128, 256:512], vA[:, ika, 2 * hcB + 1, 0:HD],
                    e2[:, 1, sl], False, last, tile_position=(0, 64), skip=True)
                for m, es in ((0, e1[:, 0, sl]), (1, e1[:, 1, sl]),
                              (2, e2[:, 0, sl]), (3, e2[:, 1, sl])):
                    _mm(nc, den[32 * m:32 * m + 1, 256:512], ones_sb[:], es,
                        False, last, tile_position=(0, 32 * m), skip=True)

            for m, (ctxt, hc, hp) in enumerate((
                    (ctxAB[0:64, :], hcA, 0), (ctxAB[64:128, :], hcA, 64),
                    (ctxCD[0:64, :], hcB, 0), (ctxCD[64:128, :], hcB, 64))):
                dst = cT[hp:hp + 64, hc, ts(j, 512)]
                denb = small_pool.tile([1, 512], F32, name="denb")
                nc.vector.tensor_copy(denb[:], den[32 * m:32 * m + 1, :])
                rec = small_pool.tile([1, 512], F32, name="rec")
                nc.vector.reciprocal_approx_fast(rec[:], denb[:])
                if dbg is not None:
                    h = 2 * hc + (1 if hp else 0)
                    nc.sync.dma_start(dbg["deno"][h:h + 1, ts(j, 512)],
                                      rec[:])
                bcs = small_pool.tile([64, 512], F32, name="bcs")
                nc.gpsimd.partition_broadcast(bcs[:], rec[:])
                nc.vector.tensor_mul(dst, ctxt, bcs[:])

        def emit_C_group(sq):
            """Output projection for one 128-row sq chunk."""
            ot = poC.tile([128, 2, 512], F16, name="ot")
            for oc in range(2):
                pp = ps_mm.tile([128, 512], F32, name="mm")
                for hcc in range(NOC):
                    _mm(nc, pp[:], cT[:, hcc, ts(sq, 128)],
                        wo[:, hcc, ts(oc, 512)],
                        hcc == 0, hcc == NOC - 1)
                nc.vector.tensor_copy(ot[:, oc, :], pp[:])
            nc.sync.dma_start(po_d[ts(sq, 128), :],
                              ot[:].rearrange("p a b -> p (a b)"))

        # A(j) projections, then attention wave j with the previous
        # wave's output-projection groups woven between head pairs
        for j in range(NSQ):
            if j > 0:
                xts[j] = xin.tile([128, NFC, 512], F16, name="xt")
                for fc in range(NFC):
                    nc.sync.dma_start(xts[j][:, fc],
                                      xT_d[ts(fc, 128), ts(j, 512)])
            for g in range(6):
                emit_A_group(j, g)
            if QUAD:
                for qd in range(2):
                    emit_quad(qd, j)
                    if j > 0:
                        emit_C_group(4 * (j - 1) + 2 * qd)
                        emit_C_group(4 * (j - 1) + 2 * qd + 1)
            else:
                for hc in range(NOC):
                    emit_pair(hc, j)
                    if j > 0:
                        emit_C_group(4 * (j - 1) + hc)
        # the last pair's normalize chain (copy/recip -> gpsimd broadcast
        # -> mul) idles the PE long enough to re-throttle the clock; a few
        # fillers keep it at 2.4 GHz so the C tail runs warm
        for i in range(6):
            fill = ps_mm.tile([128, 512], F32, name="mm")
            _mm(nc, fill[:], zw[:, 0:128], zw[:], True, True)
        for sq in range(12, 16):
            emit_C_group(sq)
        if dbg is not None:
            nc.sync.dma_start(dbg["kTo"][:], kT[:])
            nc.sync.dma_start(dbg["vAo"][:], vA[:])
            nc.sync.dma_start(dbg["ctxo"][:], cT[:])
            nc.sync.dma_start(dbg["qTo"][:], qT[:])


def make_masks():
    p = np.arange(128)[:, None]
    c = np.arange(512)[None, :]
    m0 = (c >= p).astype(np.float16)
    m1 = (c >= p + 128).astype(np.float16)
    t23 = np.concatenate([m0[:, 0:256], m1[:, 0:256]], axis=1)
    m = np.empty((128, 3, 2, 512), dtype=np.float16)
    for dd in range(2):
        m[:, 0, dd] = m0
        m[:, 1, dd] = m1
        m[:, 2, dd] = t23
    return m


def make_in_maps(x, Wq, bq, Wk, bk, Wv, bv, Wo):
    masks = make_masks()
    in_maps = []
    for c in range(NCORES):
        b, g = c // 2, c % 2
        sl = slice(g * DH, (g + 1) * DH)
        in_maps.append({
            "xT": np.ascontiguousarray(x[b].T.astype(np.float16)),
            "wq": np.ascontiguousarray(Wq[:, sl].astype(np.float16)),
            "wk": np.ascontiguousarray(Wk[:, sl].astype(np.float16)),
            "wv": np.ascontiguousarray(Wv[:, sl].astype(np.float16)),
            "wo": np.ascontiguousarray(Wo[sl, :].astype(np.float16)),
            "bqt": np.ascontiguousarray(bq[sl].reshape(NOC, 128).T),
            "bkt": np.ascontiguousarray(bk[sl].reshape(NOC, 128).T),
            "bvb": np.ascontiguousarray(
                np.broadcast_to(bv[sl].reshape(HPC, HD), (128, HPC, HD))),
            "masks": masks,
        })
    return in_maps


_CACHE = {}


def _get_program():
    if "prog" not in _CACHE:
        _CACHE["prog"] = build_program()
    return _CACHE["prog"]


def kernel(x, Wq, bq, Wk, bk, Wv, bv, Wo, bo, **run_kwargs):
    x = np.asarray(x, dtype=np.float32)
    Wq = np.asarray(Wq, dtype=np.float32)
    bq = np.asarray(bq, dtype=np.float32)
    Wk = np.asarray(Wk, dtype=np.float32)
    bk = np.asarray(bk, dtype=np.float32)
    Wv = np.asarray(Wv, dtype=np.float32)
    bv = np.asarray(bv, dtype=np.float32)
    Wo = np.asarray(Wo, dtype=np.float32)
    bo = np.asarray(bo, dtype=np.float32)

    run_kwargs.pop("f32r", None)
    nc = _get_program()
    in_maps = make_in_maps(x, Wq, bq, Wk, bk, Wv, bv, Wo)
    res = run_bass_kernel_spmd(nc, in_maps, list(range(NCORES)), **run_kwargs)
    out = np.empty((B, S, D), dtype=np.float32)
    for b in range(B):
        out[b] = (res.results[2 * b]["po"].astype(np.float32)
                  + res.results[2 * b + 1]["po"].astype(np.float32) + bo)
    _CACHE["last_results"] = res
    return out


# revision 29
# speedup vs baseline: 1.2199x; 1.0063x over previous
"""Multi-head masked self-attention on 8 TRN2 NeuronCores.

Problem: B=4, S=2048, D=1024, H=16 heads (hd=64), fp32 in/out.
  q,k,v = x@W* + b*; causal softmax(q k^T / 8) @ v; out = ctx @ Wo + bo.

Sharding: core c -> (batch b = c//2, head-group g = c%2 of 8 heads).
Each core computes a partial output projection over its 512 hidden dims;
the host sums the two partials per batch and adds bo.

Key layout/perf choices (v2, evolved from the fp32r baseline):
  - ALL matmul operands are float16 (inputs quantized host-side): PE runs
    1 elem/cell/cycle like bf16, weight loads hit the fast-weight-load
    path (~27ns vs ~187ns for fp32r 4-byte loads), DMA+SBUF traffic
    halves, and the quantization error (~1e-3 rel) is far inside the
    2e-2 gate.  PSUM accumulation stays fp32.
  - host passes xT = x[b].T; q^T, k^T computed directly as [512, S]
    (lhsT = W chunk, rhs = xT chunk); v in natural [S, 512] layout with
    a ones-column per head so the AV matmul accumulates the softmax
    denominator in PSUM row 64 for free.
  - scores are computed transposed st[sk, sq] = k q^T with K = hd = 64:
    the two heads of a column-group live in partition halves 0:64/64:128,
    so a PAIR of 64x128 PE row-tiles (tile_position (0,0)/(64,0))
    computes both heads' score blocks CONCURRENTLY in one N=512 slot,
    writing the two PSUM banks of one [128,2,512] tile.  exp runs as a
    single 1024-elem ACT call over both banks.
  - causal masking by multiplying 0/1 masks on only the sub-range of
    columns that can be masked; fully-masked blocks never computed; the
    two 256-wide diagonal chunks pack into one PSUM tile (one exp call).
  - q/k bias-add evacuations run on DVE (tensor_scalar_add), not ACT,
    keeping ACT exclusively for the exp stream that paces late waves.
  - ctx^T aliases qT's storage; output projection C(sq) groups are woven
    into the following wave so the kernel does not end with a serial
    projection tail; po is written fp16 and summed host-side.
"""

import numpy as np

import concourse.bass as bass
import concourse.mybir as mybir
import concourse.tile as tile
from concourse import bacc
from concourse.bass import ts
from concourse.bass_utils import run_bass_kernel_spmd

F32 = mybir.dt.float32
F16 = mybir.dt.float16
AF = mybir.ActivationFunctionType

B, S, D, H, HD = 4, 2048, 1024, 16, 64
G = 2                 # head groups (cores per batch)
DH = D // G           # hidden dims per core = 512
HPC = H // G          # heads per core = 8
NCORES = 8

NSQ = S // 512        # 4 sq tiles of 512
NSK = S // 128        # 16 sk chunks of 128
NFC = D // 128        # 8 feature chunks
NOC = DH // 128       # 4 out-dim chunks of the per-core hidden

WARMUP = 24           # HAM pre-warm matmuls while first DMAs land
QUAD = False          # 4-head groups: col-tiled AV pairs + 4x-tiled den
DEBUG_DUMP = False    # add kT/vA/ctx/den debug outputs to the program


def _mm(nc, out, lhsT, rhs, start, stop, tile_position=None, skip=False):
    nc.tensor.matmul(out, lhsT, rhs, start=start, stop=stop,
                     tile_position=tile_position, skip_group_check=skip)


def build_program():
    nc = bacc.Bacc("TRN2", target_bir_lowering=False, debug=False)

    xT_d = nc.dram_tensor("xT", [D, S], F16, kind="ExternalInput").ap()
    wq_d = nc.dram_tensor("wq", [D, DH], F16, kind="ExternalInput").ap()
    wk_d = nc.dram_tensor("wk", [D, DH], F16, kind="ExternalInput").ap()
    wv_d = nc.dram_tensor("wv", [D, DH], F16, kind="ExternalInput").ap()
    wo_d = nc.dram_tensor("wo", [DH, D], F16, kind="ExternalInput").ap()
    bqt_d = nc.dram_tensor("bqt", [128, NOC], F32, kind="ExternalInput").ap()
    bkt_d = nc.dram_tensor("bkt", [128, NOC], F32, kind="ExternalInput").ap()
    bvb_d = nc.dram_tensor("bvb", [128, HPC, HD], F32,
                           kind="ExternalInput").ap()
    mask_d = nc.dram_tensor("masks", [128, 3, 2, 512], F16,
                            kind="ExternalInput").ap()
    po_d = nc.dram_tensor("po", [S, D], F16, kind="ExternalOutput").ap()

    dbg = None
    if DEBUG_DUMP:
        dbg = {
            "kTo": nc.dram_tensor("kTo", [128, NOC, S], F16,
                                  kind="ExternalOutput").ap(),
            "vAo": nc.dram_tensor("vAo", [128, NSK, HPC, HD + 1], F16,
                                  kind="ExternalOutput").ap(),
            "ctxo": nc.dram_tensor("ctxo", [128, NOC, S], F16,
                                   kind="ExternalOutput").ap(),
            "deno": nc.dram_tensor("deno", [HPC, S], F32,
                                   kind="ExternalOutput").ap(),
            "qTo": nc.dram_tensor("qTo", [128, NOC, S], F16,
                                  kind="ExternalOutput").ap(),
        }

    with tile.TileContext(nc) as tc:
        _emit(tc, xT_d, wq_d, wk_d, wv_d, wo_d, bqt_d, bkt_d, bvb_d, mask_d,
              po_d, dbg)
    nc.compile()
    return nc


def _emit(tc, xT_d, wq_d, wk_d, wv_d, wo_d, bqt_d, bkt_d, bvb_d, mask_d,
          po_d, dbg=None):
    nc = tc.nc
    PS = bass.MemorySpace.PSUM

    with (
        tc.tile_pool(name="persist", bufs=1) as persist,
        tc.tile_pool(name="qkv", bufs=1) as qkv_pool,
        tc.tile_pool(name="exp", bufs=4) as exp_pool,
        tc.tile_pool(name="small", bufs=2) as small_pool,
        tc.tile_pool(name="wtsA", bufs=1) as wtsA,
        tc.tile_pool(name="xin", bufs=2) as xin,
        tc.tile_pool(name="woC", bufs=1) as woC,
        tc.tile_pool(name="poC", bufs=3) as poC,
        tc.tile_pool(name="ps_mm", bufs=(1 if QUAD else 2),
                     space=PS) as ps_mm,
        tc.tile_pool(name="ps_att", bufs=2, space=PS) as ps_att,  # 4 banks
        tc.tile_pool(name="ps_ctx", bufs=2, space=PS) as ps_ctx,  # 2 banks
        tc.tile_pool(name="ps_den", bufs=1, space=PS) as ps_den,  # 1 bank
    ):
        bqt = persist.tile([128, NOC], F32)
        bkt = persist.tile([128, NOC], F32)
        nc.sync.dma_start(bqt[:], bqt_d[:])
        nc.sync.dma_start(bkt[:], bkt_d[:])

        # HAM pre-warm: a short burst of throwaway matmuls on zeros while
        # the first x/wq DMAs land, so the PE clock ramps toward 2.4 GHz
        zw = persist.tile([128, 512], F16)
        nc.vector.memset(zw[:], 0.0)
        for i in range(WARMUP):
            pwarm = ps_mm.tile([128, 512], F32, name="mm")
            _mm(nc, pwarm[:], zw[:, 0:128], zw[:], True, True)

        # persistent activations.  qT doubles as ctx^T storage: wave j's
        # evacuation overwrites qT[:, :, j-tile] right after the last
        # score matmul that reads it (disjoint partition rows per head).
        qT = qkv_pool.tile([128, NOC, S], F16)        # q^T + bq, then ctx^T
        kT = qkv_pool.tile([128, NOC, S], F16)        # k^T + bk   [512, S]
        vA = qkv_pool.tile([128, NSK, HPC, HD + 1], F16)   # v + ones col
        nc.vector.memset(vA[:, :, :, HD:HD + 1], 1.0)
        ones_sb = persist.tile([128, 1], F16)
        nc.vector.memset(ones_sb[:], 1.0)
        # debug mode: don't alias ctx onto qT so q can be inspected
        if dbg is not None:
            cT = qkv_pool.tile([128, NOC, S], F16, name="cT")
        else:
            cT = qT

        # weights + first x tile; interleave x/wq chunks so the first
        # accumulation chain starts as soon as chunk 0 lands
        xts = {}
        xts[0] = xin.tile([128, NFC, 512], F16, name="xt")
        wq = wtsA.tile([128, NFC, DH], F16)
        wk = wtsA.tile([128, NFC, DH], F16)
        wv = wtsA.tile([128, NFC, DH], F16)
        for fc in range(NFC):
            nc.sync.dma_start(xts[0][:, fc], xT_d[ts(fc, 128), ts(0, 512)])
            nc.sync.dma_start(wq[:, fc], wq_d[ts(fc, 128), :])
        for fc in range(NFC):
            nc.sync.dma_start(wk[:, fc], wk_d[ts(fc, 128), :])
        for fc in range(NFC):
            nc.sync.dma_start(wv[:, fc], wv_d[ts(fc, 128), :])
        bvb = persist.tile([128, HPC, HD], F32)
        nc.sync.dma_start(bvb[:], bvb_d[:])
        mt = persist.tile([128, 3, 2, 512], F16)
        nc.sync.dma_start(mt[:], mask_d[:])
        wo = woC.tile([128, NOC, D], F16)
        for hc in range(NOC):
            nc.sync.dma_start(wo[:, hc], wo_d[ts(hc, 128), :])

        def emit_A_group(j, g):
            """One projection group of stage A(j): g=0..3 q/k half-pairs,
            g=4..5 v pairs.  One PSUM bank per half."""
            xt = xts[j]
            if g < 4:
                op, is_k = g // 2, g % 2
                wt, bias, dstT = ((wk, bkt, kT) if is_k else (wq, bqt, qT))
                for half in range(2):
                    oc = 2 * op + half
                    pt = ps_mm.tile([128, 512], F32, name="mm")
                    for fc in range(NFC):
                        _mm(nc, pt[:], wt[:, fc, ts(oc, 128)], xt[:, fc],
                            fc == 0, fc == NFC - 1)
                    nc.vector.tensor_scalar_add(dstT[:, oc, ts(j, 512)],
                                                pt[:], bias[:, oc:oc + 1])
            else:
                sp = g - 4
                for half in range(2):
                    sc = 2 * sp + half
                    pv = ps_mm.tile([128, 512], F32, name="mm")
                    for fc in range(NFC):
                        _mm(nc, pv[:], xt[:, fc, ts(sc, 128)], wv[:, fc],
                            fc == 0, fc == NFC - 1)
                    pv_r = pv[:].rearrange("p (h u) -> p h u", u=HD)
                    nc.vector.tensor_add(vA[:, 4 * j + sc, :, 0:HD],
                                         pv_r, bvb[:])

        def emit_pair(hc, j):
            """Attention for head pair (2hc, 2hc+1) on sq tile j.
            Scores run as concurrent 64x128 PE row-tiles (one per head),
            AV accumulates ctx^T (+denominator row) per head."""
            hA, hB = 2 * hc, 2 * hc + 1
            qA = qT[0:64, hc, ts(j, 512)]
            qB = qT[64:128, hc, ts(j, 512)]
            ctxA = ps_ctx.tile([65, 512], F32, name="ctx")
            ctxB = ps_ctx.tile([65, 512], F32, name="ctx")
            nfull = 4 * j + 2
            for ik in range(nfull):
                stp = ps_att.tile([128, 2, 512], F32, name="att")
                _mm(nc, stp[:, 0, :], kT[0:64, hc, ts(ik, 128)], qA,
                    True, True, tile_position=(0, 0))
                _mm(nc, stp[:, 1, :], kT[64:128, hc, ts(ik, 128)], qB,
                    True, True, tile_position=(64, 0))
                e = exp_pool.tile([128, 2, 512], F16, name="e")
                nc.scalar.activation(e[:], stp[:], AF.Exp, scale=0.125)
                t = ik - 4 * j
                if t == 0:
                    # only sq columns 0:128 can be masked for this chunk
                    nc.vector.tensor_mul(e[:, :, 0:128], e[:, :, 0:128],
                                         mt[:, 0, :, 0:128])
                elif t == 1:
                    nc.vector.tensor_mul(e[:, :, 0:256], e[:, :, 0:256],
                                         mt[:, 1, :, 0:256])
                _mm(nc, ctxA[:], vA[:, ik, hA, :], e[:, 0, :],
                    ik == 0, False)
                _mm(nc, ctxB[:], vA[:, ik, hB, :], e[:, 1, :],
                    ik == 0, False)
            # two 256-wide diagonal chunks (sq columns 256:512 only),
            # packed side by side into one PSUM tile -> one exp call
            ik2, ik3 = 4 * j + 2, 4 * j + 3
            qA2 = qA[:, 256:512]
            qB2 = qB[:, 256:512]
            stp = ps_att.tile([128, 2, 512], F32, name="att")
            _mm(nc, stp[:, 0, 0:256], kT[0:64, hc, ts(ik2, 128)], qA2,
                True, True, tile_position=(0, 0))
            _mm(nc, stp[:, 1, 0:256], kT[64:128, hc, ts(ik2, 128)], qB2,
                True, True, tile_position=(64, 0))
            _mm(nc, stp[:, 0, 256:512], kT[0:64, hc, ts(ik3, 128)], qA2,
                True, True, tile_position=(0, 0))
            _mm(nc, stp[:, 1, 256:512], kT[64:128, hc, ts(ik3, 128)], qB2,
                True, True, tile_position=(64, 0))
            e = exp_pool.tile([128, 2, 512], F16, name="e")
            nc.scalar.activation(e[:], stp[:], AF.Exp, scale=0.125)
            nc.vector.tensor_mul(e[:], e[:], mt[:, 2])
            _mm(nc, ctxA[:, 256:512], vA[:, ik2, hA, :], e[:, 0, 0:256],
                False, False)
            _mm(nc, ctxB[:, 256:512], vA[:, ik2, hB, :], e[:, 1, 0:256],
                False, False)
            _mm(nc, ctxA[:, 256:512], vA[:, ik3, hA, :], e[:, 0, 256:512],
                False, True)
            _mm(nc, ctxB[:, 256:512], vA[:, ik3, hB, :], e[:, 1, 256:512],
                False, True)
            if hc == 3 and j == 3:
                # kernel tail: the normalize chain (copy/recip -> gpsimd
                # broadcast -> mul) would idle the PE into a HAM
                # re-throttle; fillers PINNED on the final e tile keep the
                # clock at 2.4 GHz so the C tail runs warm
                for i in range(8):
                    fill = ps_mm.tile([128, 512], F32, name="mm")
                    _mm(nc, fill[:], e[:, 0, 0:128], zw[:], True, True)
            # normalize straight out of PSUM into qT's freed j-tile.
            # NB: reciprocal_approx_fast drops nonzero partition offsets on
            # HW and PSUM reads must be 32-aligned, so copy the den row
            # (partition 64) to a partition-0 tile first.
            for ctx, hp in ((ctxA, 0), (ctxB, 64)):
                dst = cT[hp:hp + 64, hc, ts(j, 512)]
                denb = small_pool.tile([1, 512], F32, name="denb")
                nc.vector.tensor_copy(denb[:], ctx[64:65, :])
                rec = small_pool.tile([1, 512], F32, name="rec")
                nc.vector.reciprocal_approx_fast(rec[:], denb[:])
                if dbg is not None:
                    h = 2 * hc + (1 if hp else 0)
                    nc.sync.dma_start(dbg["deno"][h:h + 1, ts(j, 512)],
                                      rec[:])
                bcs = small_pool.tile([64, 512], F32, name="bcs")
                nc.gpsimd.partition_broadcast(bcs[:], rec[:])
                nc.vector.tensor_mul(dst, ctx[0:64, :], bcs[:])

        def emit_quad(qd, j):
            """Attention for 4 heads (column groups 2qd, 2qd+1) on sq tile
            j.  Scores: concurrent 64x128 row-tile pairs per column group.
            AV: col-tiled head pairs share one ctx bank (partitions
            0:64/64:128).  Denominators: four concurrent M=1 col-tiled
            matmuls (positions 0/32/64/96) accumulate into one den bank."""
            hcA, hcB = 2 * qd, 2 * qd + 1
            ctxAB = ps_ctx.tile([128, 512], F32, name="ctx")
            ctxCD = ps_ctx.tile([128, 512], F32, name="ctx")
            den = ps_den.tile([128, 512], F32, name="den")
            qs = {hcA: (qT[0:64, hcA, ts(j, 512)], qT[64:128, hcA, ts(j, 512)]),
                  hcB: (qT[0:64, hcB, ts(j, 512)], qT[64:128, hcB, ts(j, 512)])}

            def scores_exp(hc, ik):
                qA, qB = qs[hc]
                stp = ps_att.tile([128, 2, 512], F32, name="att")
                _mm(nc, stp[:, 0, :], kT[0:64, hc, ts(ik, 128)], qA,
                    True, True, tile_position=(0, 0))
                _mm(nc, stp[:, 1, :], kT[64:128, hc, ts(ik, 128)], qB,
                    True, True, tile_position=(64, 0))
                e = exp_pool.tile([128, 2, 512], F16, name="e")
                nc.scalar.activation(e[:], stp[:], AF.Exp, scale=0.125)
                t = ik - 4 * j
                if t == 0:
                    nc.vector.tensor_mul(e[:, :, 0:128], e[:, :, 0:128],
                                         mt[:, 0, :, 0:128])
                elif t == 1:
                    nc.vector.tensor_mul(e[:, :, 0:256], e[:, :, 0:256],
                                         mt[:, 1, :, 0:256])
                return e

            nfull = 4 * j + 2
            for ik in range(nfull):
                e1 = scores_exp(hcA, ik)
                e2 = scores_exp(hcB, ik)
                st = ik == 0
                _mm(nc, ctxAB[0:64, :], vA[:, ik, 2 * hcA, 0:HD],
                    e1[:, 0, :], st, False, tile_position=(0, 0), skip=True)
                _mm(nc, ctxAB[64:128, :], vA[:, ik, 2 * hcA + 1, 0:HD],
                    e1[:, 1, :], st, False, tile_position=(0, 64), skip=True)
                _mm(nc, ctxCD[0:64, :], vA[:, ik, 2 * hcB, 0:HD],
                    e2[:, 0, :], st, False, tile_position=(0, 0), skip=True)
                _mm(nc, ctxCD[64:128, :], vA[:, ik, 2 * hcB + 1, 0:HD],
                    e2[:, 1, :], st, False, tile_position=(0, 64), skip=True)
                for m, es in ((0, e1[:, 0, :]), (1, e1[:, 1, :]),
                              (2, e2[:, 0, :]), (3, e2[:, 1, :])):
                    _mm(nc, den[32 * m:32 * m + 1, :], ones_sb[:], es,
                        st, False, tile_position=(0, 32 * m), skip=True)

            # two 256-wide diagonal chunks packed per column group
            ik2, ik3 = 4 * j + 2, 4 * j + 3
            ehi = {}
            for hc in (hcA, hcB):
                qA, qB = qs[hc]
                qA2, qB2 = qA[:, 256:512], qB[:, 256:512]
                stp = ps_att.tile([128, 2, 512], F32, name="att")
                _mm(nc, stp[:, 0, 0:256], kT[0:64, hc, ts(ik2, 128)], qA2,
                    True, True, tile_position=(0, 0))
                _mm(nc, stp[:, 1, 0:256], kT[64:128, hc, ts(ik2, 128)], qB2,
                    True, True, tile_position=(64, 0))
                _mm(nc, stp[:, 0, 256:512], kT[0:64, hc, ts(ik3, 128)], qA2,
                    True, True, tile_position=(0, 0))
                _mm(nc, stp[:, 1, 256:512], kT[64:128, hc, ts(ik3, 128)],
                    qB2, True, True, tile_position=(64, 0))
                e = exp_pool.tile([128, 2, 512], F16, name="e")
                nc.scalar.activation(e[:], stp[:], AF.Exp, scale=0.125)
                nc.vector.tensor_mul(e[:], e[:], mt[:, 2])
                ehi[hc] = e
            e1, e2 = ehi[hcA], ehi[hcB]
            for ika, sl in ((ik2, slice(0, 256)), (ik3, slice(256, 512))):
                last = ika == ik3
                _mm(nc, ctxAB[0:64, 256:512], vA[:, ika, 2 * hcA, 0:HD],
                    e1[:, 0, sl], False, last, tile_position=(0, 0),
                    skip=True)
                _mm(nc, ctxAB[64:128, 256:512], vA[:, ika, 2 * hcA + 1, 0:HD],
                    e1[:, 1, sl], False, last, tile_position=(0, 64), skip=True)
                _mm(nc, ctxCD[0:64, 256:512], vA[:, ika, 2 * hcB, 0:HD],
                    e2[:, 0, sl], False, last, tile_position=(0, 0),
                    skip=True)
                _mm(nc, ctxCD[64:128, 256:512], vA[:, ika, 2 * hcB + 1, 0:HD],
                    e2[:, 1, sl], False, last, tile_position=(0, 64), skip=True)
                for m, es in ((0, e1[:, 0, sl]), (1, e1[:, 1, sl]),
                              (2, e2[:, 0, sl]), (3, e2[:, 1, sl])):
                    _mm(nc, den[32 * m:32 * m + 1, 256:512], ones_sb[:], es,
                        False, last, tile_position=(0, 32 * m), skip=True)

            for m, (ctxt, hc, hp) in enumerate((
                    (ctxAB[0:64, :], hcA, 0), (ctxAB[64:128, :], hcA, 64),
                    (ctxCD[0:64, :], hcB, 0), (ctxCD[64:128, :], hcB, 64))):
                dst = cT[hp:hp + 64, hc, ts(j, 512)]
                denb = small_pool.tile([1, 512], F32, name="denb")
                nc.vector.tensor_copy(denb[:], den[32 * m:32 * m + 1, :])
                rec = small_pool.tile([1, 512], F32, name="rec")
                nc.vector.reciprocal_approx_fast(rec[:], denb[:])
                if dbg is not None:
                    h = 2 * hc + (1 if hp else 0)
                    nc.sync.dma_start(dbg["deno"][h:h + 1, ts(j, 512)],
                                      rec[:])
                bcs = small_pool.tile([64, 512], F32, name="bcs")
                nc.gpsimd.partition_broadcast(bcs[:], rec[:])
                nc.vector.tensor_mul(dst, ctxt, bcs[:])

        def emit_C_group(sq):
            """Output projection for one 128-row sq chunk."""
            ot = poC.tile([128, 2, 512], F16, name="ot")
            for oc in range(2):
                pp = ps_mm.tile([128, 512], F32, name="mm")
                for hcc in range(NOC):
                    _mm(nc, pp[:], cT[:, hcc, ts(sq, 128)],
                        wo[:, hcc, ts(oc, 512)],
                        hcc == 0, hcc == NOC - 1)
                nc.vector.tensor_copy(ot[:, oc, :], pp[:])
                nc.sync.dma_start(po_d[ts(sq, 128), ts(oc, 512)],
                                  ot[:, oc, :])

        # A(j) projections, then attention wave j with the previous
        # wave's output-projection groups woven between head pairs
        for j in range(NSQ):
            if j > 0:
                xts[j] = xin.tile([128, NFC, 512], F16, name="xt")
                for fc in range(NFC):
                    nc.sync.dma_start(xts[j][:, fc],
                                      xT_d[ts(fc, 128), ts(j, 512)])
            for g in range(6):
                emit_A_group(j, g)
            if QUAD:
                for qd in range(2):
                    emit_quad(qd, j)
                    if j > 0:
                        emit_C_group(4 * (j - 1) + 2 * qd)
                        emit_C_group(4 * (j - 1) + 2 * qd + 1)
            else:
                for hc in range(NOC):
                    emit_pair(hc, j)
                    if j > 0:
                        emit_C_group(4 * (j - 1) + hc)
        for sq in range(12, 16):
            emit_C_group(sq)
        if dbg is not None:
            nc.sync.dma_start(dbg["kTo"][:], kT[:])
            nc.sync.dma_start(dbg["vAo"][:], vA[:])
            nc.sync.dma_start(dbg["ctxo"][:], cT[:])
            nc.sync.dma_start(dbg["qTo"][:], qT[:])


def make_masks():
    p = np.arange(128)[:, None]
    c = np.arange(512)[None, :]
    m0 = (c >= p).astype(np.float16)
    m1 = (c >= p + 128).astype(np.float16)
    t23 = np.concatenate([m0[:, 0:256], m1[:, 0:256]], axis=1)
    m = np.empty((128, 3, 2, 512), dtype=np.float16)
    for dd in range(2):
        m[:, 0, dd] = m0
        m[:, 1, dd] = m1
        m[:, 2, dd] = t23
    return m


def make_in_maps(x, Wq, bq, Wk, bk, Wv, bv, Wo):
    masks = make_masks()
    in_maps = []
    for c in range(NCORES):
        b, g = c // 2, c % 2
        sl = slice(g * DH, (g + 1) * DH)
        in_maps.append({
            "xT": np.ascontiguousarray(x[b].T.astype(np.float16)),
            "wq": np.ascontiguousarray(Wq[:, sl].astype(np.float16)),
            "wk": np.ascontiguousarray(Wk[:, sl].astype(np.float16)),
            "wv": np.ascontiguousarray(Wv[:, sl].astype(np.float16)),
            "wo": np.ascontiguousarray(Wo[sl, :].astype(np.float16)),
            "bqt": np.ascontiguousarray(bq[sl].reshape(NOC, 128).T),
            "bkt": np.ascontiguousarray(bk[sl].reshape(NOC, 128).T),
            "bvb": np.ascontiguousarray(
                np.broadcast_to(bv[sl].reshape(HPC, HD), (128, HPC, HD))),
            "masks": masks,
        })
    return in_maps


_CACHE = {}


def _get_program():
    if "prog" not in _CACHE:
        _CACHE["prog"] = build_program()
    return _CACHE["prog"]


def kernel(x, Wq, bq, Wk, bk, Wv, bv, Wo, bo, **run_kwargs):
    x = np.asarray(x, dtype=np.float32)
    Wq = np.asarray(Wq, dtype=np.float32)
    bq = np.asarray(bq, dtype=np.float32)
    Wk = np.asarray(Wk, dtype=np.float32)
    bk = np.asarray(bk, dtype=np.float32)
    Wv = np.asarray(Wv, dtype=np.float32)
    bv = np.asarray(bv, dtype=np.float32)
    Wo = np.asarray(Wo, dtype=np.float32)
    bo = np.asarray(bo, dtype=np.float32)

    run_kwargs.pop("f32r", None)
    nc = _get_program()
    in_maps = make_in_maps(x, Wq, bq, Wk, bk, Wv, bv, Wo)
    res = run_bass_kernel_spmd(nc, in_maps, list(range(NCORES)), **run_kwargs)
    out = np.empty((B, S, D), dtype=np.float32)
    for b in range(B):
        out[b] = (res.results[2 * b]["po"].astype(np.float32)
                  + res.results[2 * b + 1]["po"].astype(np.float32) + bo)
    _CACHE["last_results"] = res
    return out
